# revision 2
# baseline (speedup 1.0000x reference)
import os
import sys
from contextlib import ExitStack

for _p in ("/opt/trn_rl_repo", "/root/.axon_site/_ro/trn_rl_repo"):
    if os.path.isdir(_p) and _p not in sys.path:
        sys.path.insert(0, _p)

import numpy as np

import concourse.bacc as bacc
import concourse.mybir as mybir
from concourse.bass_utils import run_bass_kernel_spmd
from concourse.tile import TileContext

F32 = mybir.dt.float32
BF16 = mybir.dt.bfloat16

N_CORES = 8
IMG_PER_CORE = 8
H = W = 512
HALVES = 2          # half-image = 2 row-tiles = 256 rows processed per batch
ROWS_PER_HALF = 256
FD = 2 * W          # free-dim elems per partition per half-image

# reference bit order: NW,N,NE,E = lo nibble bits 0..3; SE,S,SW,W = hi bits 0..3
# gray scale-invariant: compares unchanged under positive scaling, so compute
# g' = (w0/w2) x0 + (w1/w2) x1 + x2 with two fused scalar_tensor_tensor ops.
A_OVER_B = 0.2989 / 0.587
B_COEF = 0.587 / 0.114
W0_W2 = 0.2989 / 0.114
W1_W2 = 0.587 / 0.114

# which neighbor offsets compute their compare as GPSIMD-sub + DVE step,
# and which Horner steps (0-2 lo, 3-5 hi) run on GPSIMD
GPSIMD_SUB = (4, 5, 6)
GPSIMD_STT = ()
GPSIMD_PADS = True
GPSIMD_GRAY = True
GPSIMD_RED = True
# offsets (must be in GPSIMD_SUB) whose step runs as 2x ACT Sign instead of
# a DVE tensor_scalar: plane becomes +/-1-valued with weight 2^(i-1) and a
# constant absorbed into the is_equal bin values.
ACT_SIGN = (4, 5, 6)
# offsets with pure column shifts (E=3, W=7) whose subtract runs on the PE
# as +/-identity matmuls into PSUM, with the step via ACT Sign (always).
PE_SUB = ()

_NC_CACHE = {}


def _build(n_img=IMG_PER_CORE):
    """LBP histogram for one core: x [n_img,3,512,512] f32 -> counts [n_img,16,16].

    Per half-image (2 row-tiles folded along the free dim, FD=1024/partition):
    gray via 2 fused DVE ops (f32), row-shifted copies via SBUF DMA, 8 f32
    is_ge compares -> bf16 bit planes, nibbles via 3+3 in-place stt (bf16 2x),
    one-hot in bin-major layout via tensor_scalar is_equal (bf16 4x mode),
    then PE matmuls: lhsT/rhs [128, 8 chunks x 16 bins] accumulate diagonal
    16x16 blocks of a [128,128] PSUM tile; per-image diag-extract + reduce.
    """
    nc = bacc.Bacc(None, target_bir_lowering=False, debug=False)
    x = nc.dram_tensor("x", [n_img, 3, H, W], F32, kind="ExternalInput")
    out = nc.dram_tensor("hist", [n_img, 16, 16], F32, kind="ExternalOutput")

    ge = mybir.AluOpType.is_ge
    eq = mybir.AluOpType.is_equal
    mul = mybir.AluOpType.mult
    add = mybir.AluOpType.add

    with TileContext(nc) as tc, ExitStack() as ctx:
        cpool = ctx.enter_context(tc.tile_pool(name="const", bufs=1))
        ident = negident = None
        if PE_SUB:
            I32 = mybir.dt.int32
            io_m = cpool.tile([128, 128], I32)
            nc.gpsimd.iota(io_m[:], [[1, 128]], channel_multiplier=0)
            io_p = cpool.tile([128, 1], I32)
            nc.gpsimd.iota(io_p[:], [[0, 1]], channel_multiplier=1)
            ident = cpool.tile([128, 128], F32)
            nc.vector.tensor_tensor(
                ident[:], io_m[:], io_p[:].broadcast_to([128, 128]),
                op=mybir.AluOpType.is_equal)
            negident = cpool.tile([128, 128], F32)
            nc.vector.tensor_scalar_mul(negident[:], ident[:], -1.0)
        xpool = ctx.enter_context(tc.tile_pool(name="x", bufs=6))
        gtpool = ctx.enter_context(tc.tile_pool(name="gtmp", bufs=2))
        gpool = ctx.enter_context(tc.tile_pool(name="gray", bufs=3))
        spool = ctx.enter_context(tc.tile_pool(name="shift", bufs=2))
        dpool = ctx.enter_context(tc.tile_pool(name="dsub", bufs=1))
        bpool = ctx.enter_context(tc.tile_pool(name="bits", bufs=2))
        npool = ctx.enter_context(tc.tile_pool(name="nib", bufs=2))
        opool = ctx.enter_context(tc.tile_pool(name="onehot", bufs=1))
        rpool = ctx.enter_context(tc.tile_pool(name="red", bufs=2))
        hpool = ctx.enter_context(tc.tile_pool(name="hist", bufs=2, space="PSUM"))
        dppool = ctx.enter_context(tc.tile_pool(name="dpsum", bufs=1, space="PSUM"))

        for img in range(n_img):
            hist = hpool.tile([128, 128], F32)

            # grayscale for both halves first (shift boundary rows cross halves)
            grays = []
            for h in range(HALVES):
                g = gpool.tile([128, 2, W + 2], F32, tag="gray")
                xts = []
                for ch in range(3):
                    xt = xpool.tile([128, 2, W], F32, tag="xch")
                    src = x[img, ch, h * ROWS_PER_HALF:(h + 1) * ROWS_PER_HALF, :]
                    nc.sync.dma_start(
                        xt[:], src.rearrange("(t p) w -> p t w", p=128))
                    xts.append(xt)
                gc = g[:, :, 1:W + 1]
                if GPSIMD_GRAY:
                    # Pool has no fused scalar_tensor_tensor: use TS/TT pairs
                    t0 = gtpool.tile([128, 2, W], F32, tag="gtmp0")
                    t1 = gtpool.tile([128, 2, W], F32, tag="gtmp1")
                    nc.gpsimd.tensor_scalar_mul(t0[:], xts[0][:], W0_W2)
                    nc.gpsimd.tensor_scalar_mul(t1[:], xts[1][:], W1_W2)
                    nc.gpsimd.tensor_add(gc, t0[:], xts[2][:])
                    nc.gpsimd.tensor_add(gc, gc, t1[:])
                else:
                    nc.vector.scalar_tensor_tensor(
                        gc, xts[0][:], A_OVER_B, xts[1][:], op0=mul, op1=add)
                    nc.vector.scalar_tensor_tensor(
                        gc, gc, B_COEF, xts[2][:], op0=mul, op1=add)
                # replicate-pad columns (GPSIMD: keeps the DVE free)
                pe = nc.gpsimd if GPSIMD_PADS else nc.vector
                pe.tensor_copy(g[:, :, 0:1], g[:, :, 1:2])
                pe.tensor_copy(g[:, :, W + 1:W + 2], g[:, :, W:W + 1])
                grays.append(g)

            first_mm = True
            for h in range(HALVES):
                g = grays[h]
                up = spool.tile([128, 2, W + 2], F32, tag="up")
                dn = spool.tile([128, 2, W + 2], F32, tag="dn")
                # up[p, t] = gray row (h*256 + t*128 + p - 1), replicate at top
                nc.sync.dma_start(up[1:128, :, :], g[0:127, :, :])
                nc.sync.dma_start(up[0:1, 1, :], g[127:128, 0, :])
                if h > 0:
                    nc.sync.dma_start(up[0:1, 0, :], grays[h - 1][127:128, 1, :])
                else:
                    nc.sync.dma_start(up[0:1, 0, :], g[0:1, 0, :])
                # dn[p, t] = gray row (h*256 + t*128 + p + 1), replicate at bottom
                nc.sync.dma_start(dn[0:127, :, :], g[1:128, :, :])
                nc.sync.dma_start(dn[127:128, 0, :], g[0:1, 1, :])
                if h < HALVES - 1:
                    nc.sync.dma_start(dn[127:128, 1, :], grays[h + 1][0:1, 0, :])
                else:
                    nc.sync.dma_start(dn[127:128, 1, :], g[127:128, 1, :])

                ctr = g[:, :, 1:W + 1]
                # 8 neighbor views: (src_tile, col_offset) in reference bit
                # order. For offsets in GPSIMD_SUB, the f32 subtract runs on
                # GPSIMD and the DVE does a tensor_scalar is_ge-vs-0 step
                # (2x mode) instead of a 1x tensor_tensor compare.
                views = [
                    (up, 0), (up, 1), (up, 2), (g, 2),      # NW N NE E
                    (dn, 2), (dn, 1), (dn, 0), (g, 0),      # SE S SW W
                ]
                bs = []
                signed = []  # per offset: True if plane is +/-1-valued
                for i, (src, off) in enumerate(views):
                    b = bpool.tile([128, 2 * W], BF16, tag=f"b{i}")
                    if i in PE_SUB:
                        # d = neighbor - center via +I/-I matmuls into PSUM
                        dp = dppool.tile([128, 2, W], F32, tag=f"dp{i}")
                        for t in range(2):
                            nc.tensor.matmul(
                                dp[:, t, :], ident[:], src[:, t, off:off + W],
                                start=True, stop=False, skip_group_check=True)
                            nc.tensor.matmul(
                                dp[:, t, :], negident[:], ctr[:, t, :],
                                start=False, stop=True, skip_group_check=True)
                        s = dpool.tile([128, 2 * W], BF16, tag=f"s{i}")
                        nc.scalar.activation(
                            s[:].rearrange("p (t w) -> p t w", t=2), dp[:],
                            mybir.ActivationFunctionType.Sign)
                        nc.scalar.activation(
                            b[:], s[:], mybir.ActivationFunctionType.Sign,
                            bias=1.0, scale=2.0)
                        signed.append(True)
                        bs.append(b)
                        continue
                    if i in GPSIMD_SUB:
                        d = dpool.tile([128, 2 * W], F32, tag=f"d{i}")
                        nc.gpsimd.tensor_sub(
                            d[:].rearrange("p (t w) -> p t w", t=2),
                            src[:, :, off:off + W], ctr)
                        if i in ACT_SIGN:
                            # s = Sign(d) in {-1,0,1}; b = Sign(s+0.5) in
                            # {-1,+1}; d==0 (replicate-pad ties) lands on +1.
                            s = dpool.tile([128, 2 * W], BF16, tag=f"s{i}")
                            nc.scalar.activation(
                                s[:], d[:], mybir.ActivationFunctionType.Sign)
                            nc.scalar.activation(
                                b[:], s[:], mybir.ActivationFunctionType.Sign,
                                bias=1.0, scale=2.0)
                            signed.append(True)
                            bs.append(b)
                            continue
                        nc.vector.tensor_scalar(
                            b[:], d[:], 0.0, None, op0=ge)
                    else:
                        nc.vector.tensor_tensor(
                            b[:].rearrange("p (t w) -> p t w", t=2),
                            src[:, :, off:off + W], ctr, op=ge)
                    signed.append(False)
                    bs.append(b)

                # per-plane weight: {0,1} plane at bit k -> 2^k; +/-1 plane
                # -> 2^(k-1) with constant 2^(k-1) absorbed into bin values
                nib_lo = npool.tile([128, 2 * W], BF16, tag="lo")
                nib_hi = npool.tile([128, 2 * W], BF16, tag="hi")
                e_lo = [nc.gpsimd if k in GPSIMD_STT else nc.vector
                        for k in (0, 1, 2)]
                e_hi = [nc.gpsimd if k in GPSIMD_STT else nc.vector
                        for k in (3, 4, 5)]
                bin_off = [0.0, 0.0]
                for nib, lohi, engines in ((nib_lo, 0, e_lo), (nib_hi, 1, e_hi)):
                    planes = []
                    for k in range(4):
                        i = lohi * 4 + k
                        w = 2.0 ** (k - 1) if signed[i] else 2.0 ** k
                        if signed[i]:
                            bin_off[lohi] += 2.0 ** (k - 1)
                        planes.append((bs[i], w))
                    if not any(w == 1.0 for _, w in planes):
                        print('DEBUG signed:', signed, 'lohi', lohi,
                              'weights', [w for _, w in planes])
                    ai = next(j for j, (_, w) in enumerate(planes) if w == 1.0)
                    anchor = planes[ai][0]
                    rest = [planes[j] for j in range(4) if j != ai]
                    for si, (plane, w) in enumerate(rest):
                        prev = anchor if si == 0 else nib
                        if engines[si] is nc.gpsimd:
                            # Pool lacks fused stt: TS mul then TT add
                            tmp = npool.tile([128, 2 * W], BF16, tag="htmp")
                            nc.gpsimd.tensor_scalar_mul(tmp[:], plane[:], w)
                            nc.gpsimd.tensor_add(nib[:], tmp[:], prev[:])
                        else:
                            nc.vector.scalar_tensor_tensor(
                                nib[:], plane[:], w, prev[:],
                                op0=mul, op1=add)

                # lo one-hot grouped (g, m, c): matmul stationary needs one
                # free dim, so group 8 pixel-columns x 16 bins contiguously.
                # hi one-hot dense bin-major (guaranteed 4x DVE writes).
                ohL = opool.tile([128, (2 * W) // 8, 16, 8], BF16, tag="ohL")
                ohH = opool.tile([128, 16, 2 * W], BF16, tag="ohH")
                nib_lo_g = nib_lo[:].rearrange("p (g c) -> p g c", c=8)
                for m in range(16):
                    nc.vector.tensor_scalar(
                        ohL[:, :, m, :], nib_lo_g, float(m) - bin_off[0],
                        None, op0=eq)
                    nc.vector.tensor_scalar(
                        ohH[:, m, :], nib_hi[:], float(m) - bin_off[1],
                        None, op0=eq)

                # PE binning: 8 pixel-column chunks x 16 bins per matmul;
                # M = m_lo*8 + c, N = m_hi*8 + j, useful entries at c == j
                for g in range((2 * W) // 8):
                    last = (h == HALVES - 1) and (g == (2 * W) // 8 - 1)
                    lhsT = ohL[:, g, :, :].rearrange("p m c -> p (m c)")
                    rhs = ohH[:, :, g * 8:(g + 1) * 8]
                    nc.tensor.matmul(
                        hist[:], lhsT, rhs,
                        start=first_mm, stop=last, skip_group_check=True)
                    first_mm = False

            # reduce the 8 diagonals of the [8,8] sub-blocks:
            # hist[bl*8+j, bh*8+j] = count(lo=bl, hi=bh) within column-phase j
            hsb = rpool.tile([128, 128], F32, tag="hsb")
            nc.scalar.copy(hsb[:], hist[:])
            stack = rpool.tile([16, 8 * 16], F32, tag="stack")
            h4 = hsb[:].rearrange("(bl j) (bh k) -> bl j bh k", j=8, k=8)
            for j in range(8):
                nc.sync.dma_start(
                    stack[:, j * 16:(j + 1) * 16], h4[:, j, :, j])
            acc = rpool.tile([16, 16], F32, tag="acc")
            reng = nc.gpsimd if GPSIMD_RED else nc.vector
            reng.tensor_add(acc[:], stack[:, 0:16], stack[:, 16:32])
            for j in range(2, 8):
                reng.tensor_add(
                    acc[:], acc[:], stack[:, j * 16:(j + 1) * 16])
            nc.sync.dma_start(out[img, :, :], acc[:])

    nc.finalize()
    return nc


def _get_nc(key, n_img):
    if key not in _NC_CACHE:
        _NC_CACHE[key] = _build(n_img)
    return _NC_CACHE[key]


_LAST = {"exec_ns": None, "trace": None}


def kernel(x, _trace=False):
    x = np.ascontiguousarray(np.asarray(x), dtype=np.float32)
    bs = x.shape[0]
    n_img = bs // N_CORES
    nc = _get_nc(("v2", n_img), n_img)
    in_maps = [{"x": x[i * n_img:(i + 1) * n_img]} for i in range(N_CORES)]
    res = run_bass_kernel_spmd(
        nc, in_maps, list(range(N_CORES)), trace=_trace)
    if _trace:
        _LAST["exec_ns"] = res.exec_time_ns
        _LAST["trace"] = res.instructions_and_trace
    counts = np.concatenate(
        [res.results[i]["hist"] for i in range(N_CORES)], axis=0)
    # counts[img, lo, hi] -> hist[img, hi*16+lo]
    hist = counts.transpose(0, 2, 1).reshape(bs, 256).astype(np.float32)
    norm = np.sqrt((hist * hist).sum(axis=1, keepdims=True))
    return (hist / (norm + 1e-6)).astype(np.float32)
